# revision 4
# baseline (speedup 1.0000x reference)
"""Trainium2 Bass kernel for a per-head dense MLP (CriticCVaR head).

Computes, per head t:
    h   = silu(states[t] @ W1[t] + b1[t])        # [B, S] @ [S, H]
    out = (h @ W2[t] + b2[t]).squeeze(-1)        # [B, H] @ [H, 1] -> [B]

Sharding: heads T=32 split across 8 NeuronCores (4 heads/core, full batch).

Device layout choices:
  - states are pre-transposed on the host to [S, B] so the contraction dim
    S sits on SBUF partitions, and are fed as fp8e3 (e3m4, x2 scale): the
    PE runs fp8 moving data at bf16 rate, but the DMA stream halves to
    1 B/elem, taking the X load off the critical path. W1 stays fp16 (the
    PE allows mixed dtypes below fp32) so only X pays quantization error.
  - X is stored group-interleaved in DRAM: [NG, 128, TLOC*KCH*GCOLS], so
    each 1024-column batch group needs exactly one contiguous 1 MB DMA
    that covers all 4 heads x 2 k-chunks.
  - mm1 accumulates K=256 over 2 psum passes; silu+bias evacuates PSUM on
    the scalar engine (scale=1/XSCALE undoes the fp8 input scaling). The
    scalar engine is the steady-state pacer (~1.25 us per 1024-col silu),
    so it issues nothing but silu after startup.
  - the four heads' mm2 (M=32 each, w2 replicated x32) are col-tiled via
    tile_position onto column groups 0/32/64/96 of the PE array; the 4
    matmuls stream concurrently over separate XBUSes, so mm2 costs ~1
    matmul duration per 512-col half instead of 4.
  - startup: memset-seeded dummy matmuls trip the PE HAM activity monitor
    to 2.4 GHz while the first DMAs are in flight (they depend on no DMA);
    consts and the first X slice are split across both HWDGE rings so
    mm1/silu start ~9.5 us instead of ~14.5.
  - the last group is processed in two 512-col halves to shorten the
    final silu->mm2->evac->store dependency chain, and the last store
    rides the sync HWDGE ring (faster completion receipt than SWDGE).
"""

from contextlib import ExitStack

import numpy as np

T, B, S, H = 32, 8192, 256, 128
NCORES = 8
TLOC = T // NCORES          # heads per core
KCH = S // 128              # contraction chunks (S on partitions)
MMN = 512                   # matmul free dim (one PSUM bank of fp32)
GCOLS = 1024                # batch columns per group
XSCALE = 2.0                # host-side scale before e3m4 quantization
NWARM = 7                   # PE warm-up dummy matmuls


def build_nc(b_total: int = B, use_silu: bool = True):
    import concourse.mybir as mybir
    import concourse.tile as tile
    from concourse import bacc

    fp8 = mybir.dt.float8e3
    fp16 = mybir.dt.float16
    fp32 = mybir.dt.float32
    ng = b_total // GCOLS
    gblk = TLOC * KCH * GCOLS   # free-dim bytes of one group's X block

    nc = bacc.Bacc("TRN2", target_bir_lowering=False, debug=False)
    xb = nc.dram_tensor("xb", [ng, 128, gblk], fp8, kind="ExternalInput")
    w1 = nc.dram_tensor("w1", [128, TLOC * KCH * H], fp16, kind="ExternalInput")
    b1 = nc.dram_tensor("b1", [H, TLOC], fp32, kind="ExternalInput")
    w2 = nc.dram_tensor("w2", [H, 32 * TLOC], fp16, kind="ExternalInput")
    b2 = nc.dram_tensor("b2", [128, 1], fp32, kind="ExternalInput")  # b2[t] at row 32t
    out = nc.dram_tensor("out", [TLOC, b_total], fp32, kind="ExternalOutput")

    silu = mybir.ActivationFunctionType.Silu

    with ExitStack() as ctx:
        tc = ctx.enter_context(tile.TileContext(nc))
        cpool = ctx.enter_context(tc.tile_pool(name="const", bufs=1))
        xpool = ctx.enter_context(tc.tile_pool(name="x", bufs=3))
        zpool = ctx.enter_context(tc.tile_pool(name="z", bufs=TLOC + 4))
        spool = ctx.enter_context(tc.tile_pool(name="s", bufs=2))
        opool = ctx.enter_context(tc.tile_pool(name="o", bufs=4))
        p1pool = ctx.enter_context(tc.tile_pool(name="p1", bufs=3, space="PSUM"))
        p2pool = ctx.enter_context(tc.tile_pool(name="p2", bufs=2, space="PSUM"))

        # PE warm-up seed: memset tile, no DMA dependency. Dummy matmuls
        # on it trip the HAM activity window to full clock while the
        # const/X DMAs are still in flight.
        wseed = cpool.tile([128, MMN], fp16)
        nc.vector.memset(wseed[:, :], 0.0)
        wp = p1pool.tile([128, GCOLS], fp32, tag="p1")
        for i in range(NWARM):
            nc.tensor.matmul(
                wp[:, (i % 2) * MMN : (i % 2) * MMN + MMN],
                wseed[:, 0:H],
                wseed[:, :],
                start=True,
                stop=True,
            )

        # Consts + first X slice split across both HWDGE rings so the
        # first mm1 (needs w1 + x slice 0) and first silu (needs b1)
        # start as early as possible:
        #   sync:   w1sb | x g0 slices 1..3 | x g1..
        #   scalar: x g0 slice 0 | b1sb | w2sb | b2sb
        w1sb = cpool.tile([128, TLOC * KCH * H], fp16)
        nc.sync.dma_start(w1sb[:, :], w1.ap()[:, :])
        b1sb = cpool.tile([H, TLOC], fp32)
        w2sb = cpool.tile([H, 32 * TLOC], fp16)
        b2sb = cpool.tile([128, 1], fp32)

        # Warm-up act: preloads the Silu table during the DMA wait.
        warm_a = cpool.tile([128, MMN], fp32)
        nc.scalar.activation(
            warm_a[:, :],
            wseed[:, :],
            silu if use_silu else mybir.ActivationFunctionType.Sigmoid,
        )

        zs = {}
        for g in range(ng):
            gc = g * GCOLS
            xg = xpool.tile([128, gblk], fp8, tag="x")
            if g == 0:
                csz = gblk // 4
                # slice 0 (head 0) on the scalar ring, in parallel with
                # w1sb on sync; then scalar continues with small consts.
                nc.scalar.dma_start(xg[:, 0:csz], xb.ap()[0, :, 0:csz])
                nc.scalar.dma_start(b1sb[:, :], b1.ap()[:, :])
                nc.scalar.dma_start(w2sb[:, :], w2.ap()[:, :])
                nc.scalar.dma_start(b2sb[:, :], b2.ap()[:, :])
                for ch in range(1, 4):
                    nc.sync.dma_start(
                        xg[:, ch * csz : (ch + 1) * csz],
                        xb.ap()[0, :, ch * csz : (ch + 1) * csz],
                    )
            else:
                nc.sync.dma_start(xg[:, :], xb.ap()[g, :, :])

            # Last group: two 512-col halves to shorten the tail chain.
            halves = 2 if g == ng - 1 else 1
            hw = GCOLS // halves
            for hv in range(halves):
                hvc = hv * hw
                for t in range(TLOC):
                    p1 = p1pool.tile([128, hw], fp32, tag="p1")
                    for k in range(KCH):
                        xoff = (t * KCH + k) * GCOLS + hvc
                        for hh in range(hw // MMN):
                            hc = hh * MMN
                            nc.tensor.matmul(
                                p1[:, hc : hc + MMN],
                                w1sb[:, (t * KCH + k) * H : (t * KCH + k + 1) * H],
                                xg[:, xoff + hc : xoff + hc + MMN],
                                start=(k == 0),
                                stop=(k == KCH - 1),
                            )
                    z = zpool.tile([128, hw], fp16, tag="z")
                    if use_silu:
                        nc.scalar.activation(
                            z[:, :],
                            p1[:, :],
                            silu,
                            bias=b1sb[:, t : t + 1],
                            scale=1.0 / XSCALE,
                        )
                    else:
                        # CoreSim fallback: silu(y) = y * sigmoid(y)
                        sg = spool.tile([128, hw], fp16, tag="sg")
                        nc.scalar.activation(
                            sg[:, :],
                            p1[:, :],
                            mybir.ActivationFunctionType.Sigmoid,
                            bias=b1sb[:, t : t + 1],
                            scale=1.0 / XSCALE,
                        )
                        yb = spool.tile([128, hw], fp32, tag="yb")
                        nc.vector.tensor_scalar_mul(
                            yb[:, :], p1[:, :], 1.0 / XSCALE
                        )
                        nc.vector.tensor_scalar_add(
                            yb[:, :], yb[:, :], b1sb[:, t : t + 1]
                        )
                        nc.vector.tensor_mul(z[:, :], yb[:, :], sg[:, :])
                    zs[t] = z

                for hh in range(hw // MMN):
                    hc = hh * MMN
                    p2 = p2pool.tile([128, MMN], fp32, tag="p2")
                    for t in range(TLOC):
                        # M=32 col-tiles at column groups 32t: the 4
                        # heads' matmuls stream concurrently.
                        nc.tensor.matmul(
                            p2[32 * t : 32 * t + 32, :],
                            w2sb[:, 32 * t : 32 * t + 32],
                            zs[t][:, hc : hc + MMN],
                            start=True,
                            stop=True,
                            tile_position=(0, 32 * t),
                        )
                    o = opool.tile([128, MMN], fp32)
                    nc.vector.tensor_scalar_add(o[:, :], p2[:, :], b2sb[:, 0:1])
                    st_eng = nc.sync if g == ng - 1 else nc.gpsimd
                    st_eng.dma_start(
                        out.ap()[:, gc + hvc + hc : gc + hvc + hc + MMN],
                        o[0:97:32, :],
                    )

    nc.compile()
    return nc


def make_in_maps(states_batch, W1, b1, W2, b2):
    import ml_dtypes

    states_batch = np.asarray(states_batch)
    W1, b1, W2, b2 = (np.asarray(a) for a in (W1, b1, W2, b2))
    b_total = states_batch.shape[1]
    ng = b_total // GCOLS
    in_maps = []
    for c in range(NCORES):
        sl = slice(c * TLOC, (c + 1) * TLOC)
        xq = np.clip(states_batch[sl] * XSCALE, -15.5, 15.5).astype(
            ml_dtypes.float8_e3m4
        )
        # [TLOC, B, S] -> [TLOC, S, B] -> [TLOC, KCH, 128, NG, GCOLS]
        #   -> [NG, 128, TLOC, KCH, GCOLS] -> [NG, 128, TLOC*KCH*GCOLS]
        xbh = (
            xq.transpose(0, 2, 1)
            .reshape(TLOC, KCH, 128, ng, GCOLS)
            .transpose(3, 2, 0, 1, 4)
            .reshape(ng, 128, TLOC * KCH * GCOLS)
        )
        xbh = np.ascontiguousarray(xbh)
        w1h = (
            W1[sl]
            .reshape(TLOC, KCH, 128, H)
            .transpose(2, 0, 1, 3)
            .reshape(128, TLOC * KCH * H)
            .astype(np.float16)
        )
        b1h = np.ascontiguousarray(b1[sl].T).astype(np.float32)
        w2h = np.repeat(
            np.ascontiguousarray(W2[sl][:, :, 0].T).astype(np.float16), 32, axis=1
        )
        b2h = np.repeat(b2[sl, 0].astype(np.float32), 32).reshape(128, 1)
        in_maps.append({"xb": xbh, "w1": w1h, "b1": b1h, "w2": w2h, "b2": b2h})
    return in_maps


def run(inputs: dict, trace: bool = False):
    from concourse import bass_utils

    nc = build_nc()
    in_maps = make_in_maps(**inputs)
    res = bass_utils.run_bass_kernel_spmd(
        nc, in_maps, core_ids=list(range(NCORES)), trace=trace
    )
    out = np.concatenate([r["out"] for r in res.results], axis=0)
    return out, res


def kernel(**inputs) -> np.ndarray:
    out, _ = run(inputs)
    return out


# revision 9
# speedup vs baseline: 1.3726x; 1.3726x over previous
"""Trainium2 Bass kernel for a per-head dense MLP (CriticCVaR head).

Computes, per head t:
    h   = silu(states[t] @ W1[t] + b1[t])        # [B, S] @ [S, H]
    out = (h @ W2[t] + b2[t]).squeeze(-1)        # [B, H] @ [H, 1] -> [B]

Sharding: heads T=32 split across 8 NeuronCores (4 heads/core, full batch).

Device layout choices:
  - states are pre-transposed on the host to [S, B] so the contraction dim
    S sits on SBUF partitions, and are fed as fp8e3 (e3m4, x2 scale): the
    PE runs fp8 moving data at bf16 rate, but the DMA stream halves to
    1 B/elem, taking the X load off the critical path. W1 stays fp16 (the
    PE allows mixed dtypes below fp32) so only X pays quantization error.
  - X is stored group-interleaved in DRAM: [NG, 128, TLOC*KCH*GCOLS], so
    each 1024-column batch group needs exactly one contiguous 1 MB DMA
    that covers all 4 heads x 2 k-chunks.
  - mm1 accumulates K=256 over 2 psum passes; silu+bias evacuates PSUM on
    the scalar engine (scale=1/XSCALE undoes the fp8 input scaling). The
    scalar engine is the steady-state pacer (~1.25 us per 1024-col silu),
    so it issues nothing but silu after startup.
  - the four heads' mm2 (M=32 each, w2 replicated x32) are col-tiled via
    tile_position onto column groups 0/32/64/96 of the PE array; the 4
    matmuls stream concurrently over separate XBUSes, so mm2 costs ~1
    matmul duration per 512-col half instead of 4.
  - startup: memset-seeded dummy matmuls trip the PE HAM activity monitor
    to 2.4 GHz while the first DMAs are in flight (they depend on no DMA);
    consts and the first X slice are split across both HWDGE rings so
    mm1/silu start ~9.5 us instead of ~14.5.
  - each group's mm2 is emitted one group late so its z inputs are all
    ready when the PE reaches it (quads then issue back-to-back), and the
    last store rides the sync HWDGE ring (faster completion receipt).
"""

from contextlib import ExitStack

import numpy as np

T, B, S, H = 32, 8192, 256, 128
NCORES = 8
TLOC = T // NCORES          # heads per core
KCH = S // 128              # contraction chunks (S on partitions)
MMN = 512                   # matmul free dim (one PSUM bank of fp32)
GCOLS = 1024                # batch columns per group
XSCALE = 2.0                # host-side scale before e3m4 quantization
NWARM = 7                   # PE warm-up dummy matmuls


def build_nc(b_total: int = B, use_silu: bool = True):
    import concourse.mybir as mybir
    import concourse.tile as tile
    from concourse import bacc

    fp8 = mybir.dt.float8e3
    fp16 = mybir.dt.float16
    fp32 = mybir.dt.float32
    ng = b_total // GCOLS
    gblk = TLOC * KCH * GCOLS   # free-dim bytes of one group's X block

    nc = bacc.Bacc("TRN2", target_bir_lowering=False, debug=False)
    xb = nc.dram_tensor("xb", [ng, 128, gblk], fp8, kind="ExternalInput")
    w1 = nc.dram_tensor("w1", [128, TLOC * KCH * H], fp16, kind="ExternalInput")
    b1 = nc.dram_tensor("b1", [H, TLOC], fp32, kind="ExternalInput")
    w2 = nc.dram_tensor("w2", [H, 32 * TLOC], fp16, kind="ExternalInput")
    b2 = nc.dram_tensor("b2", [128, 1], fp32, kind="ExternalInput")  # b2[t] at row 32t
    out = nc.dram_tensor("out", [TLOC, b_total], fp32, kind="ExternalOutput")

    silu = mybir.ActivationFunctionType.Silu

    with ExitStack() as ctx:
        tc = ctx.enter_context(tile.TileContext(nc))
        cpool = ctx.enter_context(tc.tile_pool(name="const", bufs=1))
        xpool = ctx.enter_context(tc.tile_pool(name="x", bufs=3))
        zpool = ctx.enter_context(tc.tile_pool(name="z", bufs=2 * TLOC + 2))
        spool = ctx.enter_context(tc.tile_pool(name="s", bufs=2))
        opool = ctx.enter_context(tc.tile_pool(name="o", bufs=4))
        p1pool = ctx.enter_context(tc.tile_pool(name="p1", bufs=3, space="PSUM"))
        p2pool = ctx.enter_context(tc.tile_pool(name="p2", bufs=2, space="PSUM"))

        # PE warm-up seed: memset tile, no DMA dependency. Dummy matmuls
        # on it trip the HAM activity window to full clock while the
        # const/X DMAs are still in flight.
        wseed = cpool.tile([128, MMN], fp16)
        nc.vector.memset(wseed[:, :], 0.0)
        wp = p1pool.tile([128, GCOLS], fp32, tag="p1")
        for i in range(NWARM):
            nc.tensor.matmul(
                wp[:, (i % 2) * MMN : (i % 2) * MMN + MMN],
                wseed[:, 0:H],
                wseed[:, :],
                start=True,
                stop=True,
            )

        # Consts + first X slice split across both HWDGE rings so the
        # first mm1 (needs w1 + x slice 0) and first silu (needs b1)
        # start as early as possible:
        #   sync:   w1sb | x g0 slices 1..3 | x g1..
        #   scalar: x g0 slice 0 | b1sb | w2sb | b2sb
        w1sb = cpool.tile([128, TLOC * KCH * H], fp16)
        nc.sync.dma_start(w1sb[:, :], w1.ap()[:, :])
        b1sb = cpool.tile([H, TLOC], fp32)
        w2sb = cpool.tile([H, 32 * TLOC], fp16)
        b2sb = cpool.tile([128, 1], fp32)

        # Warm-up act: preloads the Silu table during the DMA wait.
        warm_a = cpool.tile([128, MMN], fp32)
        nc.scalar.activation(
            warm_a[:, :],
            wseed[:, :],
            silu if use_silu else mybir.ActivationFunctionType.Sigmoid,
        )

        zs = {}

        def emit_mm2(g):
            gc = g * GCOLS
            for hh in range(GCOLS // MMN):
                hc = hh * MMN
                p2 = p2pool.tile([128, MMN], fp32, tag="p2")
                for t in range(TLOC):
                    # M=32 col-tiles at column groups 32t: the 4 heads'
                    # matmuls stream concurrently (separate XBUSes).
                    nc.tensor.matmul(
                        p2[32 * t : 32 * t + 32, :],
                        w2sb[:, 32 * t : 32 * t + 32],
                        zs[g, t][:, hc : hc + MMN],
                        start=True,
                        stop=True,
                        tile_position=(0, 32 * t),
                    )
                o = opool.tile([128, MMN], fp32)
                nc.vector.tensor_scalar_add(o[:, :], p2[:, :], b2sb[:, 0:1])
                st_eng = nc.sync if g == ng - 1 else nc.gpsimd
                st_eng.dma_start(
                    out.ap()[:, gc + hc : gc + hc + MMN],
                    o[0:97:32, :],
                )

        for g in range(ng):
            gc = g * GCOLS
            xg = xpool.tile([128, gblk], fp8, tag="x")
            if g == 0:
                csz = gblk // 4
                # slice 0 (head 0) on the scalar ring, in parallel with
                # w1sb on sync; then scalar continues with small consts.
                nc.scalar.dma_start(xg[:, 0:csz], xb.ap()[0, :, 0:csz])
                nc.scalar.dma_start(b1sb[:, :], b1.ap()[:, :])
                nc.scalar.dma_start(w2sb[:, :], w2.ap()[:, :])
                nc.scalar.dma_start(b2sb[:, :], b2.ap()[:, :])
                for ch in range(1, 4):
                    nc.sync.dma_start(
                        xg[:, ch * csz : (ch + 1) * csz],
                        xb.ap()[0, :, ch * csz : (ch + 1) * csz],
                    )
            else:
                nc.sync.dma_start(xg[:, :], xb.ap()[g, :, :])

            for t in range(TLOC):
                p1 = p1pool.tile([128, GCOLS], fp32, tag="p1")
                for k in range(KCH):
                    xoff = (t * KCH + k) * GCOLS
                    for hh in range(GCOLS // MMN):
                        hc = hh * MMN
                        nc.tensor.matmul(
                            p1[:, hc : hc + MMN],
                            w1sb[:, (t * KCH + k) * H : (t * KCH + k + 1) * H],
                            xg[:, xoff + hc : xoff + hc + MMN],
                            start=(k == 0),
                            stop=(k == KCH - 1),
                        )
                z = zpool.tile([128, GCOLS], fp16, tag="z")
                if use_silu:
                    nc.scalar.activation(
                        z[:, :],
                        p1[:, :],
                        silu,
                        bias=b1sb[:, t : t + 1],
                        scale=1.0 / XSCALE,
                    )
                else:
                    # CoreSim fallback: silu(y) = y * sigmoid(y)
                    sg = spool.tile([128, GCOLS], fp16, tag="sg")
                    nc.scalar.activation(
                        sg[:, :],
                        p1[:, :],
                        mybir.ActivationFunctionType.Sigmoid,
                        bias=b1sb[:, t : t + 1],
                        scale=1.0 / XSCALE,
                    )
                    yb = spool.tile([128, GCOLS], fp32, tag="yb")
                    nc.vector.tensor_scalar_mul(
                        yb[:, :], p1[:, :], 1.0 / XSCALE
                    )
                    nc.vector.tensor_scalar_add(
                        yb[:, :], yb[:, :], b1sb[:, t : t + 1]
                    )
                    nc.vector.tensor_mul(z[:, :], yb[:, :], sg[:, :])
                zs[g, t] = z

            # mm2 for group g-1, deferred one group so all 4 z tiles are
            # long ready when the PE reaches these matmuls — the quads
            # then issue back-to-back and stream concurrently instead of
            # head-of-line blocking the PE queue on a missing z.
            if g >= 1:
                emit_mm2(g - 1)
        emit_mm2(ng - 1)

    nc.compile()
    return nc


def make_in_maps(states_batch, W1, b1, W2, b2):
    import ml_dtypes

    states_batch = np.asarray(states_batch)
    W1, b1, W2, b2 = (np.asarray(a) for a in (W1, b1, W2, b2))
    b_total = states_batch.shape[1]
    ng = b_total // GCOLS
    in_maps = []
    for c in range(NCORES):
        sl = slice(c * TLOC, (c + 1) * TLOC)
        xq = np.clip(states_batch[sl] * XSCALE, -15.5, 15.5).astype(
            ml_dtypes.float8_e3m4
        )
        # [TLOC, B, S] -> [TLOC, S, B] -> [TLOC, KCH, 128, NG, GCOLS]
        #   -> [NG, 128, TLOC, KCH, GCOLS] -> [NG, 128, TLOC*KCH*GCOLS]
        xbh = (
            xq.transpose(0, 2, 1)
            .reshape(TLOC, KCH, 128, ng, GCOLS)
            .transpose(3, 2, 0, 1, 4)
            .reshape(ng, 128, TLOC * KCH * GCOLS)
        )
        xbh = np.ascontiguousarray(xbh)
        w1h = (
            W1[sl]
            .reshape(TLOC, KCH, 128, H)
            .transpose(2, 0, 1, 3)
            .reshape(128, TLOC * KCH * H)
            .astype(np.float16)
        )
        b1h = np.ascontiguousarray(b1[sl].T).astype(np.float32)
        w2h = np.repeat(
            np.ascontiguousarray(W2[sl][:, :, 0].T).astype(np.float16), 32, axis=1
        )
        b2h = np.repeat(b2[sl, 0].astype(np.float32), 32).reshape(128, 1)
        in_maps.append({"xb": xbh, "w1": w1h, "b1": b1h, "w2": w2h, "b2": b2h})
    return in_maps


def run(inputs: dict, trace: bool = False):
    from concourse import bass_utils

    nc = build_nc()
    in_maps = make_in_maps(**inputs)
    res = bass_utils.run_bass_kernel_spmd(
        nc, in_maps, core_ids=list(range(NCORES)), trace=trace
    )
    out = np.concatenate([r["out"] for r in res.results], axis=0)
    return out, res


def kernel(**inputs) -> np.ndarray:
    out, _ = run(inputs)
    return out
